# revision 1
# baseline (speedup 1.0000x reference)
"""Trainium2 Bass kernel for nn_Attention2D (B=8, C=256, H=W=32, 8 heads, d=32).

Strategy: data-parallel over batch, one batch element per NeuronCore (8 cores).

Per-core pipeline (n = H*W = 1024 tokens, head dim d = 32):
  phase 0: load x [256,1024] fp32 -> bf16; load host-prepped weights.
  qkv:     q = (scale*w_q) @ x, k = w_k @ x   ([256,1024] head-major, bf16)
           vT = x^T @ w_v^T                   (8x [128,256] bf16 j-chunks)
  sim^T:   per (head, j-chunk): matmul(lhsT=k slice [32,128], rhs=q slice
           [32,512]) -> PSUM ring tiles [128,1536]; 4 heads run concurrently
           via row groups (K=32 packing). Softmax max-subtraction is skipped
           (logits ~N(0,0.8), measured max |sim| = 4.8; exp is safe in fp32).
  exp:     ACT engine Exp over PSUM ring tiles -> bf16 SBUF. This is the
           kernel roofline: 8.4M exps/core at 128/cycle @ 1.2 GHz.
  AV:      per (head pair, i-half): accumulate over j-chunks
             main psum [128,512]: headA rows 0:32 (lhsT = vT slice [128,32]),
                                  headB rows 64:96 (tile_position=(0,64))
             den  psum [128,512]: all-ones lhsT -> denominator replicated on
                                  rows 0:32 (A) / 64:96 (B)
  norm:    rc = reciprocal_approx_fast(den); out_all = main * rc (DVE),
           partition-aligned; junk rows stay zero (pre-memset out_all).
  proj:    final = w_outT_padded^T @ out_all + b_out (padded weights have zero
           rows at junk positions) -> y [256,1024] fp32.
"""

import numpy as np
import ml_dtypes

B, DIM, H, W = 8, 256, 32, 32
NUM_HEADS = 8
DIM_HEAD = 256
D = DIM_HEAD // NUM_HEADS          # 32 per-head dim
N = H * W                          # 1024 tokens
SCALE = (DIM_HEAD / NUM_HEADS) ** (-0.5)
NCORES = 8

_BF16 = ml_dtypes.bfloat16

_PROGRAM = None  # compiled Bass program cache (one per process)


def build_kernel_body(tc, y_ap, x_ap, wqkvT_ap, woutT_ap, bout_ap, dbg=None):
    """Emit the per-core attention program into TileContext tc.

    DRAM tensors:
      x_ap:     [256, 1024] fp32   (one batch element, channels x tokens)
      wqkvT_ap: [256, 768]  bf16   (w_qkv^T, q-part pre-scaled by SCALE)
      woutT_ap: [512, 256]  bf16   (w_out^T padded: 64-row blocks per head,
                                    rows 0:32 real, 32:64 zero)
      bout_ap:  [256, 1]    fp32
      y_ap:     [256, 1024] fp32 out
    """
    from contextlib import ExitStack
    from concourse import mybir

    nc = tc.nc
    f32 = mybir.dt.float32
    bf16 = mybir.dt.bfloat16

    with ExitStack() as ctx:
        singles = ctx.enter_context(tc.tile_pool(name="singles", bufs=1))
        evac = ctx.enter_context(tc.tile_pool(name="evac", bufs=2))
        exp_pool = ctx.enter_context(tc.tile_pool(name="exp", bufs=36))
        rc_pool = ctx.enter_context(tc.tile_pool(name="rc", bufs=2))
        sim_psum = ctx.enter_context(tc.tile_pool(name="simp", bufs=2, space="PSUM"))
        acc_psum = ctx.enter_context(tc.tile_pool(name="accp", bufs=4, space="PSUM"))

        # ---- phase 0: loads + conversions + constant prep ----
        xb = []
        wq = []
        for c in range(2):
            t32 = singles.tile([128, N], f32, tag=f"x32_{c}")
            nc.sync.dma_start(out=t32, in_=x_ap[c * 128:(c + 1) * 128, :])
            tb = singles.tile([128, N], bf16, tag=f"xb_{c}")
            nc.gpsimd.tensor_copy(out=tb, in_=t32)
            xb.append(tb)
            tw = singles.tile([128, 768], bf16, tag=f"wq_{c}")
            nc.sync.dma_start(out=tw, in_=wqkvT_ap[c * 128:(c + 1) * 128, :])
            wq.append(tw)
        wo = []
        for t in range(4):
            tw = singles.tile([128, 256], bf16, tag=f"wo_{t}")
            nc.sync.dma_start(out=tw, in_=woutT_ap[t * 128:(t + 1) * 128, :])
            wo.append(tw)
        bias = []
        for oc in range(2):
            tb = singles.tile([128, 1], f32, tag=f"bias_{oc}")
            nc.sync.dma_start(out=tb, in_=bout_ap[oc * 128:(oc + 1) * 128, :])
            bias.append(tb)

        ones32 = singles.tile([128, 32], bf16, tag="ones32")
        nc.gpsimd.memset(ones32, 1.0)

        # out_all: final-GEMM rhs, 4 pair tiles x [128, 1024] bf16.
        # pair p = heads (2p, 2p+1): head A rows 0:32, head B rows 64:96.
        out_all = []
        for t in range(4):
            ta = singles.tile([128, N], bf16, tag=f"out_all_{t}")
            nc.gpsimd.memset(ta, 0.0)
            out_all.append(ta)

        # ---- qkv GEMM: q (o-chunks 0,1), k (o-chunks 2,3) ----
        qk = []
        for oc in range(4):
            dst = singles.tile([128, N], bf16, tag=f"qk_{oc}")
            for nh in range(2):
                ps = acc_psum.tile([128, 512], f32, tag="acc")
                for kc in range(2):
                    nc.tensor.matmul(
                        ps,
                        wq[kc][:, oc * 128:(oc + 1) * 128],
                        xb[kc][:, nh * 512:(nh + 1) * 512],
                        start=(kc == 0),
                        stop=(kc == 1),
                    )
                nc.vector.tensor_copy(out=dst[:, nh * 512:(nh + 1) * 512], in_=ps)
            qk.append(dst)
        qb = qk[0:2]
        kb = qk[2:4]

        # ---- vT GEMM: vt[jc] = x[:, jc]^T @ w_v^T  ([128,256] bf16) ----
        vt = []
        for jc in range(8):
            ps = acc_psum.tile([128, 256], f32, tag="acc")
            for kc in range(2):
                nc.tensor.matmul(
                    ps,
                    xb[kc][:, jc * 128:(jc + 1) * 128],
                    wq[kc][:, 512:768],
                    start=(kc == 0),
                    stop=(kc == 1),
                )
            dst = singles.tile([128, 256], bf16, tag=f"vt_{jc}")
            nc.vector.tensor_copy(out=dst, in_=ps)
            vt.append(dst)

        # ---- main loop: sim^T -> exp -> AV(+den) -> normalize ----
        # production unit u = ((Q*2 + ih)*8 + jc)*4 + hq, each [128, 512].
        # ring tiles hold 3 units -> one ACT exp instruction [128, 1536].
        exp_slices = {}
        state = {"psum": None, "exp": None, "units": 0}

        def flush_group():
            if state["psum"] is None:
                return
            w = state["units"] * 512
            nc.scalar.activation(
                out=state["exp"][:, 0:w],
                in_=state["psum"][:, 0:w],
                func=mybir.ActivationFunctionType.Exp,
            )
            state["psum"] = None
            state["exp"] = None
            state["units"] = 0

        def unit_index(Q, ih, jc, hq):
            return ((Q * 2 + ih) * 8 + jc) * 4 + hq

        for Q in range(2):
            for ih in range(2):
                for jc in range(8):
                    for hq in range(4):
                        u = unit_index(Q, ih, jc, hq)
                        if state["psum"] is None:
                            state["psum"] = sim_psum.tile([128, 1024], f32, tag="sim", name=f"sim_{u}")
                            state["exp"] = exp_pool.tile([128, 1024], bf16, tag="exp", name=f"exp_{u}")
                        s = state["units"]
                        tp = (96, 0) if hq == 3 else None
                        nc.tensor.matmul(
                            state["psum"][:, s * 512:(s + 1) * 512],
                            kb[Q][32 * hq:32 * (hq + 1), jc * 128:(jc + 1) * 128],
                            qb[Q][32 * hq:32 * (hq + 1), ih * 512:(ih + 1) * 512],
                            start=True,
                            stop=True,
                            tile_position=tp,
                        )
                        exp_slices[u] = (state["exp"], s)
                        state["units"] += 1
                        if state["units"] == 2:
                            flush_group()
                if (Q, ih) == (1, 1):
                    flush_group()

                # AV + normalize for pairs of this (Q, ih)
                for pq in range(2):
                    pair = 2 * Q + pq            # heads (2*pair, 2*pair+1)
                    hA, hB = 2 * pq, 2 * pq + 1  # in-quad head indices
                    mainA = acc_psum.tile([128, 512], f32, tag="acc",
                                          name=f"mA_{pair}_{ih}")
                    mainB = acc_psum.tile([128, 512], f32, tag="acc",
                                          name=f"mB_{pair}_{ih}")
                    denA = acc_psum.tile([128, 512], f32, tag="acc",
                                         name=f"dA_{pair}_{ih}")
                    denB = acc_psum.tile([128, 512], f32, tag="acc",
                                         name=f"dB_{pair}_{ih}")
                    for jc in range(8):
                        eA, sA = exp_slices[unit_index(Q, ih, jc, hA)]
                        eB, sB = exp_slices[unit_index(Q, ih, jc, hB)]
                        rhsA = eA[:, sA * 512:(sA + 1) * 512]
                        rhsB = eB[:, sB * 512:(sB + 1) * 512]
                        st, sp = (jc == 0), (jc == 7)
                        nc.tensor.matmul(
                            mainA[0:32, :], vt[jc][:, 32 * (4 * Q + hA):32 * (4 * Q + hA) + 32],
                            rhsA, start=st, stop=sp)
                        nc.tensor.matmul(
                            mainB[64:96, :], vt[jc][:, 32 * (4 * Q + hB):32 * (4 * Q + hB) + 32],
                            rhsB, start=st, stop=sp, tile_position=(0, 64))
                        nc.tensor.matmul(
                            denA[0:32, :], ones32, rhsA, start=st, stop=sp)
                        nc.tensor.matmul(
                            denB[64:96, :], ones32, rhsB, start=st, stop=sp,
                            tile_position=(0, 64))
                    # custom-DVE ops misbehave on base_partition != 0 slices;
                    # run them over the full tile (garbage rows never read).
                    rc = rc_pool.tile([128, 512], f32, tag="rc")
                    rcB = rc_pool.tile([128, 512], f32, tag="rcB")
                    nc.vector.reciprocal_approx_fast(out=rc[:, :], in_=denA[:, :])
                    nc.vector.reciprocal_approx_fast(out=rcB[:, :], in_=denB[:, :])
                    if dbg is not None and pair == 0 and ih == 0:
                        for nm, t_, lo in (("denA", denA, 0), ("denB", denB, 64),
                                           ("mainB", mainB, 64), ("rcd", rcB, 64)):
                            if nm in dbg:
                                tmp = rc_pool.tile([128, 512], f32, tag="dbgtmp",
                                                   name=f"dbg_{nm}")
                                nc.vector.tensor_copy(out=tmp[lo:lo + 32, :],
                                                      in_=t_[lo:lo + 32, :])
                                nc.sync.dma_start(out=dbg[nm],
                                                  in_=tmp[lo:lo + 32, :])
                    dst = out_all[pair]
                    nc.vector.tensor_mul(
                        out=dst[0:32, ih * 512:(ih + 1) * 512],
                        in0=mainA[0:32, :], in1=rc[0:32, :])
                    nc.vector.tensor_mul(
                        out=dst[64:96, ih * 512:(ih + 1) * 512],
                        in0=mainB[64:96, :], in1=rcB[64:96, :])

        if dbg is not None:
            for nm, tile_ in (("qb0", qb[0]), ("qb1", qb[1]), ("kb0", kb[0]),
                              ("kb1", kb[1]), ("vt0", vt[0]), ("vt7", vt[7]),
                              ("oa0", out_all[0]), ("oa1", out_all[1]),
                              ("oa2", out_all[2]), ("oa3", out_all[3])):
                if nm in dbg:
                    nc.sync.dma_start(out=dbg[nm], in_=tile_)
            if "exp0" in dbg:
                et, s = exp_slices[unit_index(0, 0, 0, 0)]
                nc.sync.dma_start(out=dbg["exp0"], in_=et[:, s * 512:(s + 1) * 512])
            if "exp5" in dbg:
                et, s = exp_slices[unit_index(0, 0, 1, 1)]
                nc.sync.dma_start(out=dbg["exp5"], in_=et[:, s * 512:(s + 1) * 512])

        # ---- final projection + bias ----
        for oc in range(2):
            for nh in range(2):
                ps = acc_psum.tile([128, 512], f32, tag="acc")
                for t in range(4):
                    nc.tensor.matmul(
                        ps,
                        wo[t][:, oc * 128:(oc + 1) * 128],
                        out_all[t][:, nh * 512:(nh + 1) * 512],
                        start=(t == 0),
                        stop=(t == 3),
                    )
                ys = evac.tile([128, 512], f32, tag="y")
                nc.vector.tensor_scalar_add(out=ys, in0=ps, scalar1=bias[oc])
                nc.sync.dma_start(
                    out=y_ap[oc * 128:(oc + 1) * 128, nh * 512:(nh + 1) * 512],
                    in_=ys,
                )


def _prep_weights(w_qkv, w_out, b_out):
    """Host-side weight preparation (numpy)."""
    wq = w_qkv.astype(np.float32).copy()
    wq[0:DIM_HEAD] *= SCALE                      # fold softmax scale into w_q
    wqkvT = np.ascontiguousarray(wq.T).astype(_BF16)          # [256, 768]

    w_outT = np.ascontiguousarray(w_out.astype(np.float32).T)  # [hd, o]
    pad = np.zeros((8, 64, DIM), dtype=np.float32)
    for h in range(NUM_HEADS):
        pad[h, 0:D, :] = w_outT[h * D:(h + 1) * D, :]
    woutT = pad.reshape(512, DIM).astype(_BF16)               # [512, 256]

    bout = b_out.astype(np.float32).reshape(DIM, 1)           # [256, 1]
    return wqkvT, woutT, bout


def _strip_redundant_pe_waits(nc):
    """Drop transitively-implied sem waits from PE instructions.

    Walrus allows only one sync-wait command on a Matmult. Tile's semaphore
    pass is not transitively minimal: the first matmul writing a recycled
    PSUM slot waits both on the Activation exp that freed the slot AND on a
    PE tick that the exp itself already waited for. Strip wait W2 from a PE
    instruction when another wait W1 on it is served by an instruction that
    itself waited for W2's semaphore to reach at least W2's value.
    """
    for f in nc.m.functions:
        for blk in f.blocks:
            insts = list(blk.instructions)
            cum = {}
            served_by = {}  # (sem_name, cum_value) -> inst
            for ins in insts:
                if ins.sync_info is None:
                    continue
                for up in ins.sync_info.on_update:
                    if up.update_mode != "sem-inc":
                        continue
                    c = cum.get(up.ant_name, 0) + up.update_value
                    cum[up.ant_name] = c
                    served_by[(up.ant_name, c)] = ins

            def implied(w1, w2):
                # instruction completing w1 (cum hits >= w1.value first time)
                for v in range(w1.wait_value, w1.wait_value + 16):
                    srv = served_by.get((w1.ant_name, v))
                    if srv is not None:
                        break
                else:
                    return False
                srv_si = srv.sync_info
                if srv_si is None:
                    return False
                for w in srv_si.on_wait:
                    if (w.ant_name == w2.ant_name
                            and w.wait_mode == "sem-ge-imm"
                            and w.wait_value >= w2.wait_value):
                        return True
                return False

            for ins in insts:
                if str(ins.engine) not in ("EngineType.PE", "PE"):
                    continue
                si = ins.sync_info
                if si is None:
                    continue
                waits = list(si.on_wait)
                while len(waits) > 1:
                    drop = None
                    for w2 in waits:
                        if w2.wait_mode != "sem-ge-imm":
                            continue
                        for w1 in waits:
                            if w1 is w2 or w1.wait_mode != "sem-ge-imm":
                                continue
                            if implied(w1, w2):
                                drop = w2
                                break
                        if drop is not None:
                            break
                    if drop is None:
                        # Move a non-Activation wait onto the server of the
                        # first other wait: the server completes only after
                        # the moved condition, so the original ordering is
                        # preserved while this instruction keeps one wait.
                        w1 = next((w for w in waits
                                   if w.ant_name.startswith("Activation")), None)
                        w2 = next((w for w in waits if w is not w1), None)
                        if w1 is None or w2 is None:
                            break
                        srv = None
                        for v in range(w1.wait_value, w1.wait_value + 16):
                            srv = served_by.get((w1.ant_name, v))
                            if srv is not None:
                                break
                        if srv is None or srv.sync_info is None:
                            break
                        srv.sync_info.on_wait = list(srv.sync_info.on_wait) + [w2]
                        drop = w2
                    waits = [w for w in waits if w is not drop]
                if len(waits) != len(si.on_wait):
                    si.on_wait = waits
                if len(waits) > 1:
                    print(f"WARNING: {ins.name} still has {len(waits)} waits")


def _build_program():
    global _PROGRAM
    if _PROGRAM is not None:
        return _PROGRAM
    import concourse.tile as tile
    from concourse import bacc, mybir

    nc = bacc.Bacc("TRN2", target_bir_lowering=False, debug=False,
                   num_devices=NCORES)
    x_ap = nc.dram_tensor("x", [DIM, N], mybir.dt.float32,
                          kind="ExternalInput").ap()
    wqkvT_ap = nc.dram_tensor("wqkvT", [DIM, 3 * DIM_HEAD], mybir.dt.bfloat16,
                              kind="ExternalInput").ap()
    woutT_ap = nc.dram_tensor("woutT", [512, DIM], mybir.dt.bfloat16,
                              kind="ExternalInput").ap()
    bout_ap = nc.dram_tensor("bout", [DIM, 1], mybir.dt.float32,
                             kind="ExternalInput").ap()
    y_ap = nc.dram_tensor("y", [DIM, N], mybir.dt.float32,
                          kind="ExternalOutput").ap()
    with tile.TileContext(nc) as tc:
        build_kernel_body(tc, y_ap, x_ap, wqkvT_ap, woutT_ap, bout_ap)
    nc.compile()
    _PROGRAM = nc
    return nc


def kernel(x, w_qkv, w_out, b_out, trace=False):
    """Full-input entry point: shard over batch, run on 8 cores, gather."""
    from concourse import bass_utils

    nc = _build_program()
    wqkvT, woutT, bout = _prep_weights(w_qkv, w_out, b_out)
    in_maps = []
    for b in range(B):
        in_maps.append({
            "x": np.ascontiguousarray(
                np.asarray(x[b], dtype=np.float32).reshape(DIM, N)),
            "wqkvT": wqkvT,
            "woutT": woutT,
            "bout": bout,
        })
    res = bass_utils.run_bass_kernel_spmd(
        nc, in_maps, core_ids=list(range(NCORES)), trace=trace)
    y = np.stack([res.results[b]["y"].reshape(DIM, H, W) for b in range(B)])
    kernel.last_results = res
    return y



# revision 3
# speedup vs baseline: 1.2316x; 1.2316x over previous
"""Trainium2 Bass kernel for nn_Attention2D (B=8, C=256, H=W=32, 8 heads, d=32).

Strategy: data-parallel over batch, one batch element per NeuronCore (8 cores).

Per-core pipeline (n = H*W = 1024 tokens, head dim d = 32):
  phase 0: x [256,1024] fp32 -> bf16 (DVE casts); weights via DMA.
           q = (scale*w_q) @ x, k = w_k @ x  ([256,1024] head-major bf16,
           evacuated on ACT (idle pre-loop) + DVE); vT chunks on DVE.
  main loop over 64 ring tiles T (group g=(Q,ih) x jc x hq-half):
    sim^T: matmul(lhsT=k slice [32,128], rhs=q slice [32,512]) -> PSUM
           [128,1024] ring (bufs=2); 4 hq row-groups run concurrently.
    exp:   per ring tile, either ACT Exp (true exp, ~1004ns) or DVE
           Schraudolph bf16-exp (one tensor_scalar: bits =
           rne(x*128*log2e + 16256) -> int16, bitcast bf16; validated on HW:
           convert is RNE, softmax normalization cancels the ripple;
           all-approx end-to-end rel err 0.0085, mixed ~0.005).
    AV+den (lagging one jc behind sim): per (g, jc): 8 matmuls, 4-way
           column-packed: main[32h:32h+32] += vt_h @ exp_h,
           den[32h:32h+32] += ones @ exp_h (den replicated over 32 rows for
           partition-aligned normalize). 2 waves of 4 concurrent col-groups.
    norm:  rc = reciprocal_approx_fast(den); out_all[Q][:,ih] = main*rc (DVE).
  proj:  per ih half once both Q groups done: y chunk = w_out^T(Q=0,1 blocks)
         @ out_all + bias -> DMA out. No zero-padding (4 heads fill 128
         partitions exactly).
"""

import numpy as np
import ml_dtypes

B, DIM, H, W = 8, 256, 32, 32
NUM_HEADS = 8
DIM_HEAD = 256
D = DIM_HEAD // NUM_HEADS          # 32 per-head dim
N = H * W                          # 1024 tokens
SCALE = (DIM_HEAD / NUM_HEADS) ** (-0.5)
NCORES = 8

_BF16 = ml_dtypes.bfloat16

# Schraudolph bf16 exp2-trick constants: bits = rne(x*A + Bc) as int16,
# reinterpreted as bf16. A = 128*log2(e); Bc = 127*128.
EXP_A = float(128.0 * np.log2(np.e))
EXP_B = 16256.0

# Ring tiles handled by the DVE approx-exp, by within-group tile index
# (16 tiles per group). Group 0 gets fewer (DVE busy with phase-0 evacs).
DVE_TILES_G0 = {5, 8, 11, 14}
DVE_TILES = {2, 4, 7, 9, 12, 14}

_PROGRAM = None  # compiled Bass program cache (one per process)


def build_kernel_body(tc, y_ap, x_ap, wqkvT_ap, woutT_ap, bout_ap, dbg=None):
    """Emit the per-core attention program into TileContext tc.

    DRAM tensors:
      x_ap:     [256, 1024] fp32   (one batch element, channels x tokens)
      wqkvT_ap: [256, 768]  bf16   (w_qkv^T, q-part pre-scaled by SCALE)
      woutT_ap: [256, 256]  bf16   (w_out^T, head-major rows)
      bout_ap:  [256, 1]    fp32
      y_ap:     [256, 1024] fp32 out
    """
    from contextlib import ExitStack
    from concourse import mybir

    nc = tc.nc
    f32 = mybir.dt.float32
    bf16 = mybir.dt.bfloat16
    i16 = mybir.dt.int16

    with ExitStack() as ctx:
        singles = ctx.enter_context(tc.tile_pool(name="singles", bufs=1))
        evac = ctx.enter_context(tc.tile_pool(name="evac", bufs=2))
        exp_pool = ctx.enter_context(tc.tile_pool(name="exp", bufs=12))
        rc_pool = ctx.enter_context(tc.tile_pool(name="rc", bufs=2))
        sim_psum = ctx.enter_context(tc.tile_pool(name="simp", bufs=2, space="PSUM"))
        acc_psum = ctx.enter_context(tc.tile_pool(name="accp", bufs=4, space="PSUM"))

        # ---- phase 0: DMA loads ----
        xs = []
        xb = []
        for c in range(2):
            t32 = singles.tile([128, N], f32, tag=f"x32_{c}")
            for h in range(2):
                nc.sync.dma_start(out=t32[:, h * 512:(h + 1) * 512],
                                  in_=x_ap[c * 128:(c + 1) * 128,
                                           h * 512:(h + 1) * 512])
            xs.append(t32)
        wq = []
        for c in range(2):
            tw = singles.tile([128, 768], bf16, tag=f"wq_{c}")
            # k columns first (sim needs k earliest), then q, then v
            for lo, hi in ((256, 512), (0, 256), (512, 768)):
                nc.sync.dma_start(out=tw[:, lo:hi],
                                  in_=wqkvT_ap[c * 128:(c + 1) * 128, lo:hi])
            wq.append(tw)
        wo = []
        for q in range(2):
            tw = singles.tile([128, 256], bf16, tag=f"wo_{q}")
            nc.sync.dma_start(out=tw, in_=woutT_ap[q * 128:(q + 1) * 128, :])
            wo.append(tw)
        bias = []
        for oc in range(2):
            tb = singles.tile([128, 1], f32, tag=f"bias_{oc}")
            nc.sync.dma_start(out=tb, in_=bout_ap[oc * 128:(oc + 1) * 128, :])
            bias.append(tb)

        ones32 = singles.tile([128, 32], bf16, tag="ones32")
        nc.gpsimd.memset(ones32, 1.0)

        # x fp32 -> bf16 on DVE (fast 2x_2P mode, startup-critical)
        for c in range(2):
            tb = singles.tile([128, N], bf16, tag=f"xb_{c}")
            nc.vector.tensor_copy(out=tb, in_=xs[c])
            xb.append(tb)

        # out_all[Q]: normalized attention output, 4 heads stacked on
        # partitions, [128, 1024] bf16. Fully written before proj reads.
        out_all = []
        for q in range(2):
            ta = singles.tile([128, N], bf16, tag=f"out_all_{q}")
            out_all.append(ta)

        # ---- qkv GEMM: emit k0, q0, k1, q1 (oc = 2, 0, 3, 1) ----
        # Early evacs go to ACT (idle before the exp stream starts).
        qk = {}
        act_evacs = {(2, 0), (2, 1), (0, 0)}
        for oc in (2, 0, 3, 1):
            dst = singles.tile([128, N], bf16, tag=f"qk_{oc}")
            for nh in range(2):
                ps = acc_psum.tile([128, 512], f32, tag="acc")
                for kc in range(2):
                    nc.tensor.matmul(
                        ps,
                        wq[kc][:, oc * 128:(oc + 1) * 128],
                        xb[kc][:, nh * 512:(nh + 1) * 512],
                        start=(kc == 0),
                        stop=(kc == 1),
                    )
                out_slice = dst[:, nh * 512:(nh + 1) * 512]
                if (oc, nh) in act_evacs:
                    nc.scalar.activation(out=out_slice, in_=ps,
                                         func=mybir.ActivationFunctionType.Copy)
                else:
                    nc.vector.tensor_copy(out=out_slice, in_=ps)
            qk[oc] = dst
        qb = [qk[0], qk[1]]
        kb = [qk[2], qk[3]]

        # ---- vT GEMM: vt[jc] = x[:, jc]^T @ w_v^T  ([128,256] bf16) ----
        vt = []
        for jc in range(8):
            ps = acc_psum.tile([128, 256], f32, tag="acc")
            for kc in range(2):
                nc.tensor.matmul(
                    ps,
                    xb[kc][:, jc * 128:(jc + 1) * 128],
                    wq[kc][:, 512:768],
                    start=(kc == 0),
                    stop=(kc == 1),
                )
            dst = singles.tile([128, 256], bf16, tag=f"vt_{jc}")
            nc.vector.tensor_copy(out=dst, in_=ps)
            vt.append(dst)

        # ---- main loop: globally pipelined sim -> exp -> AV -> norm ----
        # groups (Q, ih) in ih-outer order so proj chunk ih fires when both
        # Q-halves of that token range are normalized.
        groups = [(0, 0), (1, 0), (0, 1), (1, 1)]
        exp_ref = {}   # (gi, jc, hq) -> (exp_tile, slice)
        av_main = {}
        av_den = {}

        def emit_sim_exp(Jg):
            gi, jc = Jg // 8, Jg % 8
            Q, ih = groups[gi]
            for half in range(2):
                t_in = 2 * (Jg % 8) + half      # within-group tile index
                T = gi * 16 + t_in
                st = sim_psum.tile([128, 1024], f32, tag="sim", name=f"sim_{T}")
                for s in range(2):
                    hq = 2 * half + s
                    tp = (96, 0) if hq == 3 else None
                    nc.tensor.matmul(
                        st[:, s * 512:(s + 1) * 512],
                        kb[Q][32 * hq:32 * (hq + 1), jc * 128:(jc + 1) * 128],
                        qb[Q][32 * hq:32 * (hq + 1), ih * 512:(ih + 1) * 512],
                        start=True,
                        stop=True,
                        tile_position=tp,
                    )
                    exp_ref[(gi, jc, hq)] = (None, s)  # placeholder
                et = exp_pool.tile([128, 1024], bf16, tag="exp", name=f"exp_{T}")
                dve_set = DVE_TILES_G0 if gi == 0 else DVE_TILES
                if t_in in dve_set:
                    nc.vector.tensor_scalar(
                        out=et[:, :].bitcast(i16),
                        in0=st[:, :],
                        scalar1=EXP_A,
                        scalar2=EXP_B,
                        op0=mybir.AluOpType.mult,
                        op1=mybir.AluOpType.add,
                    )
                else:
                    nc.scalar.activation(
                        out=et, in_=st,
                        func=mybir.ActivationFunctionType.Exp)
                for s in range(2):
                    exp_ref[(gi, jc, 2 * half + s)] = (et, s)

        def emit_av(Jg):
            gi, jc = Jg // 8, Jg % 8
            Q, ih = groups[gi]
            if jc == 0:
                av_main[gi] = acc_psum.tile([128, 512], f32, tag="acc",
                                            name=f"main_{gi}")
                av_den[gi] = acc_psum.tile([128, 512], f32, tag="acc",
                                           name=f"den_{gi}")
            st, sp = (jc == 0), (jc == 7)
            main_t, den_t = av_main[gi], av_den[gi]
            for hq in range(4):
                et, s = exp_ref[(gi, jc, hq)]
                rhs = et[:, s * 512:(s + 1) * 512]
                tp = (0, 96) if hq == 3 else None
                nc.tensor.matmul(
                    main_t[32 * hq:32 * (hq + 1), :],
                    vt[jc][:, 32 * (4 * Q + hq):32 * (4 * Q + hq) + 32],
                    rhs, start=st, stop=sp, tile_position=tp)
            for hq in range(4):
                et, s = exp_ref[(gi, jc, hq)]
                rhs = et[:, s * 512:(s + 1) * 512]
                tp = (0, 96) if hq == 3 else None
                nc.tensor.matmul(
                    den_t[32 * hq:32 * (hq + 1), :],
                    ones32, rhs, start=st, stop=sp, tile_position=tp)

        def finish_group(gi):
            Q, ih = groups[gi]
            rc = rc_pool.tile([128, 512], f32, tag="rc", name=f"rc_{gi}")
            nc.vector.reciprocal_approx_fast(out=rc, in_=av_den[gi])
            nc.vector.tensor_mul(
                out=out_all[Q][:, ih * 512:(ih + 1) * 512],
                in0=av_main[gi], in1=rc)
            if dbg is not None and gi == 0:
                for nm, t_ in (("den0", av_den[gi]), ("main0", av_main[gi]),
                               ("rc0", rc)):
                    if nm in dbg:
                        tmp = rc_pool.tile([128, 512], f32, tag="dbgtmp",
                                           name=f"dbg_{nm}")
                        nc.vector.tensor_copy(out=tmp, in_=t_)
                        nc.sync.dma_start(out=dbg[nm], in_=tmp)
            if Q == 1:
                emit_proj(ih)

        def emit_proj(ih):
            for oc in range(2):
                ps = acc_psum.tile([128, 512], f32, tag="acc",
                                   name=f"proj_{ih}_{oc}")
                for Qp in range(2):
                    nc.tensor.matmul(
                        ps,
                        wo[Qp][:, oc * 128:(oc + 1) * 128],
                        out_all[Qp][:, ih * 512:(ih + 1) * 512],
                        start=(Qp == 0),
                        stop=(Qp == 1),
                    )
                ys = evac.tile([128, 512], f32, tag="y")
                nc.vector.tensor_scalar_add(out=ys, in0=ps, scalar1=bias[oc])
                nc.sync.dma_start(
                    out=y_ap[oc * 128:(oc + 1) * 128, ih * 512:(ih + 1) * 512],
                    in_=ys,
                )

        for J in range(32):
            emit_sim_exp(J)
            if J >= 1:
                emit_av(J - 1)
                if (J - 1) % 8 == 7:
                    finish_group((J - 1) // 8)
        emit_av(31)
        finish_group(3)

        if dbg is not None:
            for nm, tile_ in (("qb0", qb[0]), ("qb1", qb[1]), ("kb0", kb[0]),
                              ("kb1", kb[1]), ("vt0", vt[0]), ("vt7", vt[7]),
                              ("oa0", out_all[0]), ("oa1", out_all[1])):
                if nm in dbg:
                    nc.sync.dma_start(out=dbg[nm], in_=tile_)
            if "exp0" in dbg:
                et, s = exp_ref[(0, 0, 0)]
                nc.sync.dma_start(out=dbg["exp0"],
                                  in_=et[:, s * 512:(s + 1) * 512])


def _prep_weights(w_qkv, w_out, b_out):
    """Host-side weight preparation (numpy)."""
    wq = w_qkv.astype(np.float32).copy()
    wq[0:DIM_HEAD] *= SCALE                      # fold softmax scale into w_q
    wqkvT = np.ascontiguousarray(wq.T).astype(_BF16)            # [256, 768]
    woutT = np.ascontiguousarray(
        w_out.astype(np.float32).T).astype(_BF16)               # [256, 256]
    bout = b_out.astype(np.float32).reshape(DIM, 1)             # [256, 1]
    return wqkvT, woutT, bout


def _strip_redundant_pe_waits(nc):
    """Drop transitively-implied sem waits from PE instructions.

    Walrus allows only one sync-wait command on a Matmult. Tile's semaphore
    pass is not transitively minimal: the first matmul writing a recycled
    PSUM slot waits both on the Activation exp that freed the slot AND on a
    PE tick that the exp itself already waited for. Strip wait W2 from a PE
    instruction when another wait W1 on it is served by an instruction that
    itself waited for W2's semaphore to reach at least W2's value.
    """
    for f in nc.m.functions:
        for blk in f.blocks:
            insts = list(blk.instructions)
            cum = {}
            served_by = {}  # (sem_name, cum_value) -> inst
            for ins in insts:
                if ins.sync_info is None:
                    continue
                for up in ins.sync_info.on_update:
                    if up.update_mode != "sem-inc":
                        continue
                    c = cum.get(up.ant_name, 0) + up.update_value
                    cum[up.ant_name] = c
                    served_by[(up.ant_name, c)] = ins

            def implied(w1, w2):
                # instruction completing w1 (cum hits >= w1.value first time)
                for v in range(w1.wait_value, w1.wait_value + 16):
                    srv = served_by.get((w1.ant_name, v))
                    if srv is not None:
                        break
                else:
                    return False
                srv_si = srv.sync_info
                if srv_si is None:
                    return False
                for w in srv_si.on_wait:
                    if (w.ant_name == w2.ant_name
                            and w.wait_mode == "sem-ge-imm"
                            and w.wait_value >= w2.wait_value):
                        return True
                return False

            for ins in insts:
                if str(ins.engine) not in ("EngineType.PE", "PE"):
                    continue
                si = ins.sync_info
                if si is None:
                    continue
                waits = list(si.on_wait)
                while len(waits) > 1:
                    drop = None
                    for w2 in waits:
                        if w2.wait_mode != "sem-ge-imm":
                            continue
                        for w1 in waits:
                            if w1 is w2 or w1.wait_mode != "sem-ge-imm":
                                continue
                            if implied(w1, w2):
                                drop = w2
                                break
                        if drop is not None:
                            break
                    if drop is None:
                        # Move a non-Activation wait onto the server of the
                        # first other wait: the server completes only after
                        # the moved condition, so the original ordering is
                        # preserved while this instruction keeps one wait.
                        w1 = next((w for w in waits
                                   if w.ant_name.startswith("Activation")), None)
                        if w1 is None:
                            w1 = next((w for w in waits
                                       if w.ant_name.startswith("Vector")), None)
                        w2 = next((w for w in waits if w is not w1), None)
                        if w1 is None or w2 is None:
                            break
                        srv = None
                        for v in range(w1.wait_value, w1.wait_value + 16):
                            srv = served_by.get((w1.ant_name, v))
                            if srv is not None:
                                break
                        if srv is None or srv.sync_info is None:
                            break
                        srv.sync_info.on_wait = list(srv.sync_info.on_wait) + [w2]
                        drop = w2
                    waits = [w for w in waits if w is not drop]
                if len(waits) != len(si.on_wait):
                    si.on_wait = waits
                if len(waits) > 1:
                    print(f"WARNING: {ins.name} still has {len(waits)} waits")


def _build_program():
    global _PROGRAM
    if _PROGRAM is not None:
        return _PROGRAM
    import concourse.tile as tile
    from concourse import bacc, mybir

    nc = bacc.Bacc("TRN2", target_bir_lowering=False, debug=False,
                   num_devices=NCORES)
    x_ap = nc.dram_tensor("x", [DIM, N], mybir.dt.float32,
                          kind="ExternalInput").ap()
    wqkvT_ap = nc.dram_tensor("wqkvT", [DIM, 3 * DIM_HEAD], mybir.dt.bfloat16,
                              kind="ExternalInput").ap()
    woutT_ap = nc.dram_tensor("woutT", [DIM_HEAD, DIM], mybir.dt.bfloat16,
                              kind="ExternalInput").ap()
    bout_ap = nc.dram_tensor("bout", [DIM, 1], mybir.dt.float32,
                             kind="ExternalInput").ap()
    y_ap = nc.dram_tensor("y", [DIM, N], mybir.dt.float32,
                          kind="ExternalOutput").ap()
    with tile.TileContext(nc) as tc:
        build_kernel_body(tc, y_ap, x_ap, wqkvT_ap, woutT_ap, bout_ap)
    nc.compile()
    _PROGRAM = nc
    return nc


def kernel(x, w_qkv, w_out, b_out, trace=False):
    """Full-input entry point: shard over batch, run on 8 cores, gather."""
    from concourse import bass_utils

    nc = _build_program()
    wqkvT, woutT, bout = _prep_weights(w_qkv, w_out, b_out)
    in_maps = []
    for b in range(B):
        in_maps.append({
            "x": np.ascontiguousarray(
                np.asarray(x[b], dtype=np.float32).reshape(DIM, N)),
            "wqkvT": wqkvT,
            "woutT": woutT,
            "bout": bout,
        })
    res = bass_utils.run_bass_kernel_spmd(
        nc, in_maps, core_ids=list(range(NCORES)), trace=trace)
    y = np.stack([res.results[b]["y"].reshape(DIM, H, W) for b in range(B)])
    kernel.last_results = res
    return y
